# revision 24
# baseline (speedup 1.0000x reference)
"""CAFE-interpolation kernel for 8 Trainium2 NeuronCores.

Strategy: shard the T axis (1024 = 8 x 128) across cores. Every core holds a
T-slice of ALL 128 samples, so the sr[partner_idx] gather is core-local.

Math: with mask_b = (im_b > thr_b) in {0,1}^D and c_b = is_dominant_b*(1-m_b):

  out[b] = x[b] + c_b * ( mask[p_b] . x[p_b] - mask[b] . x[b] )

Only dominant rows differ from x, so the device returns just those rows
(packed via the matmul's stationary gather matrix); the host assembles
out = x.copy() and scatters the device rows in.

Per-core pipeline (inputs are fp16, host-converted; halves read traffic and
enables the DVE 2x 16-bit mode):

  stage 1: im_partial[b,d] = sum_{t in slice} g*x. DVE: fp16 product +
           pairwise tree-add (fp16), f32 accumulation across t-groups on
           GpSimd. x tiles stay resident in SBUF for stage 3 (16 MB).
  AllReduce im_partial [128, 512] fp16 across 8 cores (~128 KB); staged
           via HWDGE 128-row DMAs; a tiny dummy AllReduce early in stage
           1 keeps the CC cores armed.
  stage 2: exact 52nd/53rd largest per row via 7 rounds of the DVE max-8
           instruction + match_replace (top-k extraction, 8 ranks/round);
           thr = v459 + 0.9*(v460-v459) exactly like jnp.quantile (the
           1/T mean scale cancels: mask is scale-invariant).
  stage 3: per t-group: xm = x * mask (DVE, fp16); PSUM accumulates
           A^T@x + Pc^T@xm where A packs dominant rows and Pc = c*(P - I);
           Act/DVE copy PSUM->fp16 SBUF; 128-row fp16 DMA stores (rows
           past n_dom are zero-padding so every store spreads across all
           16 DMA engines).

The same program works for every (partner_idx, is_dominant, mixup): the
metadata enters only through the amat/pmat input tensors; one compile
serves any input.
"""

import os
import numpy as np

B, T, D = 128, 1024, 512
N_CORES = 8
T_LOC = T // N_CORES  # 128
TG1 = 8  # stage-1 t-steps per group (16 groups)
TG3 = 4  # stage-3 t-steps per group (32 groups); [n_dom, 4*512] f32 = 4 PSUM banks
NSEL = 7  # max-8 rounds: ranks 1..56 cover v460 (rank 52) and v459 (rank 53)

_CACHE: dict = {}
LAST_RESULT = None


def _build():
    import concourse.mybir as mybir
    import concourse.tile as tile
    from concourse import bacc

    f32 = mybir.dt.float32
    f16 = mybir.dt.float16
    Alu = mybir.AluOpType

    _dbg = os.environ.get("KBUILD_DEBUG") == "1"

    nc = bacc.Bacc(
        "TRN2", target_bir_lowering=False, debug=False, num_devices=N_CORES
    )
    x_sl = nc.dram_tensor("x_sl", [B, T_LOC, D], f16, kind="ExternalInput")
    g_sl = nc.dram_tensor("g_sl", [B, T_LOC, D], f16, kind="ExternalInput")
    # stationary matrices are padded to the full 128 columns (zeros past
    # n_dom) so every store DMA carries 128 partitions -- patterns with
    # fewer rows get pinned to a single DMA engine instead of the 16-way
    # split (measured: 71-row stores drained at 21 GB/s on one engine).
    amat_in = nc.dram_tensor("amat", [B, B], f16, kind="ExternalInput")
    pmat_in = nc.dram_tensor("pmat", [B, B], f16, kind="ExternalInput")
    out_sl = nc.dram_tensor("out_sl", [B, T_LOC, D], f16, kind="ExternalOutput")
    if _dbg:
        dbg_im = nc.dram_tensor("dbg_im", [B, D], f32, kind="ExternalOutput")
        dbg_mask = nc.dram_tensor("dbg_mask", [B, D], f32, kind="ExternalOutput")

    n_g1 = T_LOC // TG1

    with tile.TileContext(nc) as tc:
        with tc.tile_pool(name="persist", bufs=1) as pp:
            amat_t = pp.tile([B, B], f16)
            nc.sync.dma_start(amat_t[:], amat_in[:])
            pmat_t = pp.tile([B, B], f16)
            nc.sync.dma_start(pmat_t[:], pmat_in[:])

            # persistent x cache: 16 tiles of [128, 8, 512] fp16 (16 MB)
            xts = [pp.tile([B, TG1, D], f16, name=f"xc{i}") for i in range(n_g1)]

            imacc = pp.tile([B, D], f32)
            im_all = pp.tile([B, D], f16)
            sel_a = pp.tile([B, D], f16)
            sel_b = pp.tile([B, D], f16)
            mv = pp.tile([B, 8 * NSEL], f16)
            mask3 = pp.tile([B, 1, D], f16)
            thr_t = pp.tile([B, 1], f32)
            d1 = pp.tile([B, 1], f32)
            wdump = pp.tile([B, 8], f16)

            # ---- stage 1: im_partial = sum_t x*g ----
            # last 8 t-steps run as two half-groups so the end-of-stage
            # serial DVE tree tail is ~2us instead of ~4us
            chunks = [(i * TG1, TG1) for i in range(n_g1 - 1)]
            chunks += [((n_g1 - 1) * TG1, TG1 // 2),
                       ((n_g1 - 1) * TG1 + TG1 // 2, TG1 // 2)]
            with (
                tc.tile_pool(name="gld", bufs=2) as gld,
                tc.tile_pool(name="wk1", bufs=2) as wk1,
                tc.tile_pool(name="ccp", bufs=1, space="DRAM") as ccp,
            ):
                for i, (t0, tg) in enumerate(chunks):
                    xdst = xts[t0 // TG1][:, t0 % TG1 : t0 % TG1 + tg, :]
                    nc.sync.dma_start(xdst, x_sl[:, t0 : t0 + tg, :])
                    gt = gld.tile([B, tg, D], f16, tag=f"g1_{tg}")
                    nc.sync.dma_start(gt[:], g_sl[:, t0 : t0 + tg, :])
                    prod = wk1.tile([B, tg, D], f16, tag=f"prod_{tg}")
                    nc.vector.tensor_tensor(prod[:], xdst, gt[:], op=Alu.mult)
                    l1 = wk1.tile([B, tg // 2, D], f16, tag=f"l1_{tg}")
                    nc.vector.tensor_tensor(
                        l1[:], prod[:, 0 : tg // 2, :], prod[:, tg // 2 :, :],
                        op=Alu.add,
                    )
                    l2 = wk1.tile([B, tg // 4, D], f16, tag=f"l2_{tg}")
                    nc.vector.tensor_tensor(
                        l2[:], l1[:, 0 : tg // 4, :], l1[:, tg // 4 :, :],
                        op=Alu.add,
                    )
                    if tg == 4:
                        l3s = l2[:, 0, :]  # [B, D] already
                    else:
                        l3 = wk1.tile([B, D], f16, tag="l3")
                        nc.vector.tensor_tensor(
                            l3[:], l2[:, 0, :], l2[:, 1, :], op=Alu.add
                        )
                        l3s = l3[:]
                    if i == 0:
                        nc.vector.tensor_copy(imacc[:], l3s)
                        # tiny dummy AllReduce early in stage 1: keeps the
                        # CC cores armed so the real collective's pickup
                        # latency (measured up to ~15us cold) shrinks
                        ccw_in = ccp.tile([B, 8], f16, name="ccw_in")
                        ccw_out = ccp.tile([B, 8], f16, name="ccw_out")
                        nc.scalar.dma_start(ccw_in[:], amat_t[:, 0:8])
                        nc.gpsimd.collective_compute(
                            "AllReduce",
                            Alu.add,
                            replica_groups=[list(range(N_CORES))],
                            ins=[ccw_in.opt()],
                            outs=[ccw_out.opt()],
                        )
                        nc.scalar.dma_start(wdump[:], ccw_out[:])
                    else:
                        # accumulate on GpSimd to keep DVE free
                        nc.gpsimd.tensor_tensor(
                            imacc[:], imacc[:], l3s, op=Alu.add
                        )

                # ---- AllReduce the partial importance (no 1/T scale:
                # quantile mask is scale-invariant). fp16 payload: the CC
                # runs at ~11 GB/s algorithmic bw, so halving the bytes
                # cuts its transfer phase; fp16 rounding of the partials
                # moves the final rel err only ~1e-3 (host-simulated).
                # Staging DMAs go via HWDGE (scalar queue): the gpsimd
                # software queue runs the copy on a single DMA engine. ----
                im16 = pp.tile([B, D], f16)
                nc.vector.tensor_copy(im16[:], imacc[:])
                cc_in_t = ccp.tile([B, D], f16, name="cc_in_t")
                cc_out_t = ccp.tile([B, D], f16, name="cc_out_t")
                nc.scalar.dma_start(cc_in_t[:], im16[:])
                nc.gpsimd.collective_compute(
                    "AllReduce",
                    Alu.add,
                    replica_groups=[list(range(N_CORES))],
                    ins=[cc_in_t.opt()],
                    outs=[cc_out_t.opt()],
                )
                nc.scalar.dma_start(im_all[:], cc_out_t[:])

            # ---- stage 2: ranks 52/53 via 8-wide max extraction ----
            with tc.tile_pool(name="psumw", bufs=1, space="PSUM") as psumw:
                cur = im_all
                for r in range(NSEL):
                    nc.vector.max(mv[:, 8 * r : 8 * r + 8], cur[:])
                    if r < NSEL - 1:
                        nxt = sel_a if r % 2 == 0 else sel_b
                        nc.vector.match_replace(
                            nxt[:], mv[:, 8 * r : 8 * r + 8], cur[:], 0.0
                        )
                        cur = nxt

                # PE p-state warm-up (junk matmuls; scheduler places them
                # wherever deps allow)
                qw = psumw.tile([B, D], f32)
                for _ in range(20):
                    nc.tensor.matmul(
                        qw[:], amat_t[:], xts[0][:, 0, :], start=True, stop=True
                    )

                # thr = v459 + 0.9*(v460 - v459); v460 = rank 52, v459 = rank 53
                nc.vector.tensor_tensor(
                    d1[:], mv[:, 51:52], mv[:, 52:53], op=Alu.subtract
                )
                nc.vector.scalar_tensor_tensor(
                    thr_t[:], d1[:], 0.9, mv[:, 52:53], op0=Alu.mult, op1=Alu.add
                )
                nc.vector.tensor_scalar(
                    mask3[:, 0, :],
                    im_all[:],
                    scalar1=thr_t[:, 0:1],
                    scalar2=None,
                    op0=Alu.is_gt,
                )
                if _dbg:
                    nc.gpsimd.dma_start(dbg_im[:], im_all[:])
                    dbgm = pp.tile([B, D], f32)
                    nc.vector.tensor_copy(dbgm[:], mask3[:, 0, :])
                    nc.gpsimd.dma_start(dbg_mask[:], dbgm[:])

            # ---- stage 3: psum = A^T@x + Pc^T@(x*mask); Act copies psum
            # -> fp16 SBUF; DMA fp16 -> out ----
            with (
                tc.tile_pool(name="xmp", bufs=3) as xmp,
                tc.tile_pool(name="otp", bufs=4) as otp,
                tc.tile_pool(name="qp", bufs=2, space="PSUM") as qp,
            ):
                for gi, t0 in enumerate(range(0, T_LOC, TG3)):
                    xti = xts[t0 // TG1]
                    s0 = t0 % TG1
                    xs = xti[:, s0 : s0 + TG3, :]
                    xm = xmp.tile([B, TG3, D], f16, tag="xm")
                    nc.vector.tensor_tensor(
                        xm[:], xs, mask3[:].to_broadcast([B, TG3, D]), op=Alu.mult
                    )
                    q = qp.tile([B, TG3, D], f32, tag="q")
                    for j in range(TG3):
                        nc.tensor.matmul(
                            q[:, j, :], amat_t[:], xs[:, j, :],
                            start=True, stop=False,
                        )
                        nc.tensor.matmul(
                            q[:, j, :], pmat_t[:], xm[:, j, :],
                            start=False, stop=True,
                        )
                    ot = otp.tile([B, TG3, D], f16, tag="ot")
                    # psum->fp16 copies mostly on Act, a few on DVE so the
                    # Act engine is not the lone stage-3 pole (GpSimd cannot
                    # read PSUM)
                    if gi % 5 == 2:
                        nc.vector.tensor_copy(ot[:], q[:])
                    else:
                        nc.scalar.copy(ot[:], q[:])
                    nc.sync.dma_start(out_sl[:, t0 : t0 + TG3, :], ot[:])
    nc.compile()
    return nc


def kernel(x, scenario_gradient, mixup_strength, scenario, partner_idx, is_dominant):
    global LAST_RESULT
    from concourse.bass_utils import run_bass_kernel_spmd

    x = np.ascontiguousarray(np.asarray(x, dtype=np.float32))
    dm = np.asarray(is_dominant, dtype=bool).ravel()
    dom = np.flatnonzero(dm)
    n_dom = int(dom.size)
    if n_dom == 0:
        return x.copy()

    g = np.ascontiguousarray(np.asarray(scenario_gradient, dtype=np.float32))
    m = np.asarray(mixup_strength, dtype=np.float32).ravel()
    p = np.asarray(partner_idx, dtype=np.int64).ravel()

    nc = _CACHE.get("main")
    if nc is None:
        nc = _build()
        _CACHE["main"] = nc

    # stationary matrices: amat gathers dominant rows; pmat = c*(P - I);
    # columns n_dom..127 stay zero (output rows ignored by the host)
    j = np.arange(n_dom)
    amat = np.zeros((B, B), dtype=np.float16)
    amat[dom, j] = 1.0
    c = (1.0 - m[dom]).astype(np.float32)
    pmat = np.zeros((B, B), dtype=np.float32)
    np.add.at(pmat, (p[dom], j), c)
    np.add.at(pmat, (dom, j), -c)
    pmat16 = pmat.astype(np.float16)

    x16 = x.astype(np.float16)
    g16 = g.astype(np.float16)

    in_maps = []
    for ci in range(N_CORES):
        sl = slice(ci * T_LOC, (ci + 1) * T_LOC)
        in_maps.append(
            {
                "x_sl": np.ascontiguousarray(x16[:, sl, :]),
                "g_sl": np.ascontiguousarray(g16[:, sl, :]),
                "amat": amat,
                "pmat": pmat16,
            }
        )

    res = run_bass_kernel_spmd(nc, in_maps, core_ids=list(range(N_CORES)))
    LAST_RESULT = res

    out = x.copy()
    for ci in range(N_CORES):
        out[dom, ci * T_LOC : (ci + 1) * T_LOC, :] = res.results[ci]["out_sl"][:n_dom]
    return out


# revision 27
# speedup vs baseline: 1.0630x; 1.0630x over previous
"""CAFE-interpolation kernel for 8 Trainium2 NeuronCores.

Strategy: shard the T axis (1024 = 8 x 128) across cores. Every core holds a
T-slice of ALL 128 samples, so the sr[partner_idx] gather is core-local.

Math: with mask_b = (im_b > thr_b) in {0,1}^D and c_b = is_dominant_b*(1-m_b):

  out[b] = x[b] + c_b * ( mask[p_b] . x[p_b] - mask[b] . x[b] )

Only dominant rows differ from x, so the device returns just those rows
(packed via the matmul's stationary gather matrix); the host assembles
out = x.copy() and scatters the device rows in.

Per-core pipeline (inputs are fp16, host-converted; halves read traffic and
enables the DVE 2x 16-bit mode):

  stage 1: im_partial[b,d] = sum_{t in slice} g*x. DVE: fp16 product +
           pairwise tree-add (fp16), f32 accumulation across t-groups on
           GpSimd. x tiles stay resident in SBUF for stage 3 (16 MB).
  AllReduce im_partial [128, 512] fp16 across 8 cores (~128 KB); staged
           via HWDGE 128-row DMAs.
  stage 2: exact 52nd/53rd largest per row via 7 rounds of the DVE max-8
           instruction + match_replace (top-k extraction, 8 ranks/round);
           thr = v459 + 0.9*(v460-v459) exactly like jnp.quantile (the
           1/T mean scale cancels: mask is scale-invariant).
  stage 3: per t-group: xm = x * mask (DVE, fp16); PSUM accumulates
           A^T@x + Pc^T@xm where A packs dominant rows and Pc = c*(P - I);
           Act/DVE copy PSUM->fp16 SBUF; 128-row fp16 DMA stores (rows
           past n_dom are zero-padding so every store spreads across all
           16 DMA engines).

The same program works for every (partner_idx, is_dominant, mixup): the
metadata enters only through the amat/pmat input tensors; one compile
serves any input.
"""

import os
import numpy as np

B, T, D = 128, 1024, 512
N_CORES = 8
T_LOC = T // N_CORES  # 128
TG1 = 8  # stage-1 t-steps per group (16 groups)
TG3 = 4  # stage-3 t-steps per group (32 groups); [n_dom, 4*512] f32 = 4 PSUM banks
NSEL = 7  # max-8 rounds: ranks 1..56 cover v460 (rank 52) and v459 (rank 53)

_CACHE: dict = {}
LAST_RESULT = None


def _build():
    import concourse.mybir as mybir
    import concourse.tile as tile
    from concourse import bacc

    f32 = mybir.dt.float32
    f16 = mybir.dt.float16
    Alu = mybir.AluOpType

    _dbg = os.environ.get("KBUILD_DEBUG") == "1"

    nc = bacc.Bacc(
        "TRN2", target_bir_lowering=False, debug=False, num_devices=N_CORES
    )
    x_sl = nc.dram_tensor("x_sl", [B, T_LOC, D], f16, kind="ExternalInput")
    g_sl = nc.dram_tensor("g_sl", [B, T_LOC, D], f16, kind="ExternalInput")
    # stationary matrices are padded to the full 128 columns (zeros past
    # n_dom) so every store DMA carries 128 partitions -- patterns with
    # fewer rows get pinned to a single DMA engine instead of the 16-way
    # split (measured: 71-row stores drained at 21 GB/s on one engine).
    amat_in = nc.dram_tensor("amat", [B, B], f16, kind="ExternalInput")
    pmat_in = nc.dram_tensor("pmat", [B, B], f16, kind="ExternalInput")
    out_sl = nc.dram_tensor("out_sl", [B, T_LOC, D], f16, kind="ExternalOutput")
    if _dbg:
        dbg_im = nc.dram_tensor("dbg_im", [B, D], f32, kind="ExternalOutput")
        dbg_mask = nc.dram_tensor("dbg_mask", [B, D], f32, kind="ExternalOutput")

    n_g1 = T_LOC // TG1

    with tile.TileContext(nc) as tc:
        with tc.tile_pool(name="persist", bufs=1) as pp:
            amat_t = pp.tile([B, B], f16)
            nc.sync.dma_start(amat_t[:], amat_in[:])
            pmat_t = pp.tile([B, B], f16)
            nc.sync.dma_start(pmat_t[:], pmat_in[:])

            # persistent x cache: 16 tiles of [128, 8, 512] fp16 (16 MB)
            xts = [pp.tile([B, TG1, D], f16, name=f"xc{i}") for i in range(n_g1)]

            imacc = pp.tile([B, D], f32)
            im_all = pp.tile([B, D], f16)
            sel_a = pp.tile([B, D], f16)
            sel_b = pp.tile([B, D], f16)
            mv = pp.tile([B, 8 * NSEL], f16)
            mask3 = pp.tile([B, 1, D], f16)
            thr_t = pp.tile([B, 1], f32)
            d1 = pp.tile([B, 1], f32)

            # ---- stage 1: im_partial = sum_t x*g ----
            # last 8 t-steps run as two half-groups so the end-of-stage
            # serial DVE tree tail is ~2us instead of ~4us
            chunks = [(i * TG1, TG1) for i in range(n_g1 - 1)]
            chunks += [((n_g1 - 1) * TG1, TG1 // 2),
                       ((n_g1 - 1) * TG1 + TG1 // 2, TG1 // 2)]
            with (
                tc.tile_pool(name="gld", bufs=2) as gld,
                tc.tile_pool(name="wk1", bufs=2) as wk1,
                tc.tile_pool(name="ccp", bufs=1, space="DRAM") as ccp,
            ):
                for i, (t0, tg) in enumerate(chunks):
                    xdst = xts[t0 // TG1][:, t0 % TG1 : t0 % TG1 + tg, :]
                    nc.sync.dma_start(xdst, x_sl[:, t0 : t0 + tg, :])
                    gt = gld.tile([B, tg, D], f16, tag=f"g1_{tg}")
                    nc.sync.dma_start(gt[:], g_sl[:, t0 : t0 + tg, :])
                    prod = wk1.tile([B, tg, D], f16, tag=f"prod_{tg}")
                    nc.vector.tensor_tensor(prod[:], xdst, gt[:], op=Alu.mult)
                    l1 = wk1.tile([B, tg // 2, D], f16, tag=f"l1_{tg}")
                    nc.vector.tensor_tensor(
                        l1[:], prod[:, 0 : tg // 2, :], prod[:, tg // 2 :, :],
                        op=Alu.add,
                    )
                    l2 = wk1.tile([B, tg // 4, D], f16, tag=f"l2_{tg}")
                    nc.vector.tensor_tensor(
                        l2[:], l1[:, 0 : tg // 4, :], l1[:, tg // 4 :, :],
                        op=Alu.add,
                    )
                    if tg == 4:
                        l3s = l2[:, 0, :]  # [B, D] already
                    else:
                        l3 = wk1.tile([B, D], f16, tag="l3")
                        nc.vector.tensor_tensor(
                            l3[:], l2[:, 0, :], l2[:, 1, :], op=Alu.add
                        )
                        l3s = l3[:]
                    if i == 0:
                        nc.vector.tensor_copy(imacc[:], l3s)
                    else:
                        # accumulate on GpSimd to keep DVE free
                        nc.gpsimd.tensor_tensor(
                            imacc[:], imacc[:], l3s, op=Alu.add
                        )

                # ---- AllReduce the partial importance (no 1/T scale:
                # quantile mask is scale-invariant). fp16 payload: the CC
                # runs at ~11 GB/s algorithmic bw, so halving the bytes
                # cuts its transfer phase; fp16 rounding of the partials
                # moves the final rel err only ~1e-3 (host-simulated).
                # Staging DMAs go via HWDGE (scalar queue): the gpsimd
                # software queue runs the copy on a single DMA engine. ----
                im16 = pp.tile([B, D], f16)
                nc.vector.tensor_copy(im16[:], imacc[:])
                cc_in_t = ccp.tile([B, D], f16, name="cc_in_t")
                cc_out_t = ccp.tile([B, D], f16, name="cc_out_t")
                nc.scalar.dma_start(cc_in_t[:], im16[:])
                nc.gpsimd.collective_compute(
                    "AllReduce",
                    Alu.add,
                    replica_groups=[list(range(N_CORES))],
                    ins=[cc_in_t.opt()],
                    outs=[cc_out_t.opt()],
                )
                nc.scalar.dma_start(im_all[:], cc_out_t[:])

            # ---- stage 2: ranks 52/53 via 8-wide max extraction ----
            with tc.tile_pool(name="psumw", bufs=1, space="PSUM") as psumw:
                cur = im_all
                for r in range(NSEL):
                    nc.vector.max(mv[:, 8 * r : 8 * r + 8], cur[:])
                    if r < NSEL - 1:
                        nxt = sel_a if r % 2 == 0 else sel_b
                        nc.vector.match_replace(
                            nxt[:], mv[:, 8 * r : 8 * r + 8], cur[:], 0.0
                        )
                        cur = nxt

                # PE p-state warm-up (junk matmuls; scheduler places them
                # wherever deps allow)
                qw = psumw.tile([B, D], f32)
                for _ in range(20):
                    nc.tensor.matmul(
                        qw[:], amat_t[:], xts[0][:, 0, :], start=True, stop=True
                    )

                # thr = v459 + 0.9*(v460 - v459); v460 = rank 52, v459 = rank 53
                nc.vector.tensor_tensor(
                    d1[:], mv[:, 51:52], mv[:, 52:53], op=Alu.subtract
                )
                nc.vector.scalar_tensor_tensor(
                    thr_t[:], d1[:], 0.9, mv[:, 52:53], op0=Alu.mult, op1=Alu.add
                )
                nc.vector.tensor_scalar(
                    mask3[:, 0, :],
                    im_all[:],
                    scalar1=thr_t[:, 0:1],
                    scalar2=None,
                    op0=Alu.is_gt,
                )
                if _dbg:
                    nc.gpsimd.dma_start(dbg_im[:], im_all[:])
                    dbgm = pp.tile([B, D], f32)
                    nc.vector.tensor_copy(dbgm[:], mask3[:, 0, :])
                    nc.gpsimd.dma_start(dbg_mask[:], dbgm[:])

            # ---- stage 3: psum = A^T@x + Pc^T@(x*mask); Act copies psum
            # -> fp16 SBUF; DMA fp16 -> out ----
            with (
                tc.tile_pool(name="xmp", bufs=3) as xmp,
                tc.tile_pool(name="otp", bufs=4) as otp,
                tc.tile_pool(name="qp", bufs=2, space="PSUM") as qp,
            ):
                for gi, t0 in enumerate(range(0, T_LOC, TG3)):
                    xti = xts[t0 // TG1]
                    s0 = t0 % TG1
                    xs = xti[:, s0 : s0 + TG3, :]
                    xm = xmp.tile([B, TG3, D], f16, tag="xm")
                    nc.vector.tensor_tensor(
                        xm[:], xs, mask3[:].to_broadcast([B, TG3, D]), op=Alu.mult
                    )
                    q = qp.tile([B, TG3, D], f32, tag="q")
                    for j in range(TG3):
                        nc.tensor.matmul(
                            q[:, j, :], amat_t[:], xs[:, j, :],
                            start=True, stop=False,
                        )
                        nc.tensor.matmul(
                            q[:, j, :], pmat_t[:], xm[:, j, :],
                            start=False, stop=True,
                        )
                    ot = otp.tile([B, TG3, D], f16, tag="ot")
                    # psum->fp16 copies mostly on Act, a few on DVE so the
                    # Act engine is not the lone stage-3 pole (GpSimd cannot
                    # read PSUM)
                    if gi % 5 == 2:
                        nc.vector.tensor_copy(ot[:], q[:])
                    else:
                        nc.scalar.copy(ot[:], q[:])
                    nc.sync.dma_start(out_sl[:, t0 : t0 + TG3, :], ot[:])
    nc.compile()
    return nc


def kernel(x, scenario_gradient, mixup_strength, scenario, partner_idx, is_dominant):
    global LAST_RESULT
    from concourse.bass_utils import run_bass_kernel_spmd

    x = np.ascontiguousarray(np.asarray(x, dtype=np.float32))
    dm = np.asarray(is_dominant, dtype=bool).ravel()
    dom = np.flatnonzero(dm)
    n_dom = int(dom.size)
    if n_dom == 0:
        return x.copy()

    g = np.ascontiguousarray(np.asarray(scenario_gradient, dtype=np.float32))
    m = np.asarray(mixup_strength, dtype=np.float32).ravel()
    p = np.asarray(partner_idx, dtype=np.int64).ravel()

    nc = _CACHE.get("main")
    if nc is None:
        nc = _build()
        _CACHE["main"] = nc

    # stationary matrices: amat gathers dominant rows; pmat = c*(P - I);
    # columns n_dom..127 stay zero (output rows ignored by the host)
    j = np.arange(n_dom)
    amat = np.zeros((B, B), dtype=np.float16)
    amat[dom, j] = 1.0
    c = (1.0 - m[dom]).astype(np.float32)
    pmat = np.zeros((B, B), dtype=np.float32)
    np.add.at(pmat, (p[dom], j), c)
    np.add.at(pmat, (dom, j), -c)
    pmat16 = pmat.astype(np.float16)

    x16 = x.astype(np.float16)
    g16 = g.astype(np.float16)

    in_maps = []
    for ci in range(N_CORES):
        sl = slice(ci * T_LOC, (ci + 1) * T_LOC)
        in_maps.append(
            {
                "x_sl": np.ascontiguousarray(x16[:, sl, :]),
                "g_sl": np.ascontiguousarray(g16[:, sl, :]),
                "amat": amat,
                "pmat": pmat16,
            }
        )

    res = run_bass_kernel_spmd(nc, in_maps, core_ids=list(range(N_CORES)))
    LAST_RESULT = res

    out = x.copy()
    for ci in range(N_CORES):
        out[dom, ci * T_LOC : (ci + 1) * T_LOC, :] = res.results[ci]["out_sl"][:n_dom]
    return out
